# revision 2
# baseline (speedup 1.0000x reference)
"""Single-head attention (B=4, S=2048, F=1024) on 8 TRN2 NeuronCores.

All matmuls run as fp32r (11-bit-mantissa fp32; full PE rate). The Q
projection is algebraically fused into the key side:

  logits = (q Wq^T + bq)(k Wk^T + bk)^T
         = q (Wq^T Wk) k^T + (per-row const) + c[t],   c[t] = bq . kp[t]

The per-row term drops out of softmax, so with W* = Wq^T Wk and
ke = k @ W*^T the device computes logits0 = q . ke^T (raw q streamed
from DRAM, no Q projection matmuls at all) and applies the per-key bias
multiplicatively on the probabilities: esc' = exp(logits0 - m) * E,
E[t] = e^{c[t]} (host-precomputed, applied on the vector engine).

Key-split sharding: core c handles batch b=c//2 and KEY half h=c%2
(keys [h*1024, (h+1)*1024)), with ALL 2048 q rows. Each core emits an
UNNORMALIZED partial attention output plus per-row (max, sum) softmax
stats; the host merges the two halves flash-style:
  m = max(m0, m1); out = (o0*e^{m0-m} + o1*e^{m1-m}) /
                         (s0*e^{m0-m} + s1*e^{m1-m}) + q + bv
"""

import numpy as np
from contextlib import ExitStack

import concourse.bass as bass
import concourse.tile as tile
import concourse.mybir as mybir
from concourse import bacc
from concourse.bass_utils import run_bass_kernel_spmd
from concourse.masks import make_identity


B, S, F = 4, 2048, 1024
P = 128
SK = S // 2            # keys per core
FT = F // P            # 8 contraction tiles
GT = F // P            # 8 output-feature tiles
KC = SK // 512         # 2 key chunks of 512
QI = S // P            # 16 q-tiles per core
KB = SK // P           # 8 key blocks
N_CORES = 8

f32 = mybir.dt.float32
f32r = mybir.dt.float32r
bf16 = mybir.dt.bfloat16
fp16 = mybir.dt.float16
AX = mybir.AxisListType.X
AF = mybir.ActivationFunctionType
ALU = mybir.AluOpType

_CACHE = {}


def _build(repeat=1):
    nc = bacc.Bacc("TRN2", target_bir_lowering=False, debug=False,
                   num_devices=N_CORES)
    qT = nc.dram_tensor("qT", [F, S], fp16, kind="ExternalInput").ap()
    kT = nc.dram_tensor("kT", [F, SK], fp16, kind="ExternalInput").ap()
    vT = nc.dram_tensor("vT", [F, SK], fp16, kind="ExternalInput").ap()
    wsT = nc.dram_tensor("wsT", [F, F], fp16, kind="ExternalInput").ap()
    wvT = nc.dram_tensor("wvT", [F, F], fp16, kind="ExternalInput").ap()
    Eb = nc.dram_tensor("Eb", [P, SK], f32, kind="ExternalInput").ap()
    out = nc.dram_tensor("out", [S, F], f32, kind="ExternalOutput").ap()
    ms = nc.dram_tensor("ms", [S, 2], f32, kind="ExternalOutput").ap()

    with tile.TileContext(nc) as tc, ExitStack() as ctx:
      consts = ctx.enter_context(tc.tile_pool(name="consts", bufs=1))
      wpool = ctx.enter_context(tc.tile_pool(name="w", bufs=8))
      xin = ctx.enter_context(tc.tile_pool(name="xin", bufs=16))
      vxin = ctx.enter_context(tc.tile_pool(name="vxin", bufs=16))
      qx_pool = ctx.enter_context(tc.tile_pool(name="qx", bufs=2))
      proj = ctx.enter_context(tc.tile_pool(name="proj", bufs=1))
      sm = ctx.enter_context(tc.tile_pool(name="sm", bufs=2))
      stats = ctx.enter_context(tc.tile_pool(name="stats", bufs=2))
      outp = ctx.enter_context(tc.tile_pool(name="outp", bufs=2))
      psA = ctx.enter_context(tc.tile_pool(name="psA", bufs=4, space="PSUM"))
      psT = ctx.enter_context(tc.tile_pool(name="psT", bufs=2, space="PSUM"))
      psV = ctx.enter_context(tc.tile_pool(name="psV", bufs=2, space="PSUM"))
      for _rep in range(repeat):
        ident = consts.tile([P, P], fp16, tag="ident")
        make_identity(nc, ident)
        keT = [proj.tile([P, SK], fp16, tag=f"keT{g}", name=f"keT{g}")
               for g in range(GT)]
        vp = [proj.tile([P, F], fp16, tag=f"vp{i}", name=f"vp{i}")
              for i in range(KB)]

        # DMA issue order = need order: interleave ws and kx(sc=0) tiles so
        # the ft-outer first chunk can start after ws[0]+kx[0] land; wv/E/qx
        # follow, they are needed tens of us later.
        vx01 = [vxin.tile([P, 512], fp16, tag="vxin", name="vxin")
                for _ in range(2 * FT)]
        wv_sb = [wpool.tile([P, F], fp16, tag="wv", name="wv")
                 for _ in range(FT)]
        for ft in range(FT):
            nc.sync.dma_start(wv_sb[ft][:],
                              wvT[ft * P:(ft + 1) * P, :])
            nc.sync.dma_start(
                vx01[ft][:], vT[ft * P:(ft + 1) * P, 0:512])
        wsA = [wpool.tile([P, 512], fp16, tag="wsA", name="wsA")
               for _ in range(FT)]
        wsB = [wpool.tile([P, 512], fp16, tag="wsB", name="wsB")
               for _ in range(FT)]
        kx0 = [xin.tile([P, 512], fp16, tag="xin", name="xin")
               for _ in range(FT)]
        for ft in range(FT):
            nc.sync.dma_start(wsA[ft][:],
                              wsT[ft * P:(ft + 1) * P, 0:512])
            nc.sync.dma_start(
                kx0[ft][:],
                kT[ft * P:(ft + 1) * P, 0:512])
        for ft in range(FT):
            nc.sync.dma_start(wsB[ft][:],
                              wsT[ft * P:(ft + 1) * P, 512:1024])
        for ft in range(FT):
            nc.sync.dma_start(
                vx01[FT + ft][:], vT[ft * P:(ft + 1) * P, 512:1024])
        kx1 = [xin.tile([P, 512], fp16, tag="xin", name="xin")
               for _ in range(FT)]
        for ft in range(FT):
            nc.sync.dma_start(
                kx1[ft][:],
                kT[ft * P:(ft + 1) * P, 512:1024])
        E_sb = consts.tile([P, SK], f32, tag="Eb")
        nc.sync.dma_start(E_sb[:], Eb)

        # ---- projections, interleaved V/ke per key chunk so DMA
        # arrives ahead of need: V0 (3MB bf16), ke0 (ws+kx0 stream), V1, ke1.
        # All ft-outer in half-chunks of 4 psum tiles: first matmul needs only
        # the ft=0 tiles; the rest stream in behind compute, and the scalar
        # engine drains each half-chunk while the PE runs the next.
        def vproj_chunk(sc):
            vx = vx01[sc * FT:(sc + 1) * FT]
            for half in range(2):
                psh = [psA.tile([P, 512], f32, tag="mmps", name="psh")
                       for _ in range(4)]
                combos = [(half * 2 + b, gc) for b in range(2)
                          for gc in range(2)]
                for ft in range(FT):
                    for ci, (blk, gc) in enumerate(combos):
                        nc.tensor.matmul(
                            psh[ci][:], vx[ft][:, blk * P:(blk + 1) * P],
                            wv_sb[ft][:, gc * 512:(gc + 1) * 512],
                            start=(ft == 0), stop=(ft == FT - 1))
                for ci, (blk, gc) in enumerate(combos):
                    kb = sc * 4 + blk
                    nc.vector.tensor_copy(vp[kb][:, gc * 512:(gc + 1) * 512],
                                          psh[ci][:])

        def keproj_chunk(sc, kxc):
            for hc in range(2):
                wsh = wsA if hc == 0 else wsB
                psh = [psA.tile([P, 512], f32, tag="mmps", name="psh")
                       for _ in range(4)]
                for ft in range(FT):
                    for gi in range(4):
                        nc.tensor.matmul(psh[gi][:],
                                         wsh[ft][:, gi * P:(gi + 1) * P],
                                         kxc[ft][:], start=(ft == 0),
                                         stop=(ft == FT - 1))
                for gi in range(4):
                    gt = hc * 4 + gi
                    nc.scalar.activation(keT[gt][:, sc * 512:(sc + 1) * 512],
                                         psh[gi][:], AF.Identity, scale=1.0)

        vproj_chunk(0)
        keproj_chunk(0, kx0)
        vproj_chunk(1)
        keproj_chunk(1, kx1)

        # ---- attention over the local key half, pipelined over q-tiles ----
        # qx groups hold 4 q-tiles (2KB DMA lines); rotate via pool bufs=2.
        def load_qx(qg):
            qx = [qx_pool.tile([P, 512], fp16, tag=f"qx{ft}", name="qx")
                  for ft in range(FT)]
            for ft in range(FT):
                nc.sync.dma_start(
                    qx[ft][:],
                    qT[ft * P:(ft + 1) * P, qg * 512:(qg + 1) * 512])
            return qx

        def emit_logits(qx, qi):
            o = (qi % 4) * P
            lps = []
            m4 = stats.tile([P, KC], f32, tag="m4")
            for kc in range(KC):
                ps = psA.tile([P, 512], f32, tag="mmps")
                for gt in range(GT):
                    nc.tensor.matmul(ps[:], qx[gt][:, o:o + P],
                                     keT[gt][:, kc * 512:(kc + 1) * 512],
                                     start=(gt == 0), stop=(gt == GT - 1))
                nc.vector.reduce_max(m4[:, kc:kc + 1], ps[:], axis=AX)
                lps.append(ps)
            return lps, m4

        qxg = load_qx(0)
        nxt_qxg = None
        pend = None
        cur = emit_logits(qxg, 0)
        for qi in range(QI):
            if qi % 4 == 0 and qi // 4 + 1 < QI // 4:
                nxt_qxg = load_qx(qi // 4 + 1)
            lps, m4 = cur
            negm = stats.tile([P, 1], f32, tag="negm")
            nc.vector.reduce_max(negm[:], m4[:], axis=AX, negate=True)
            esc = sm.tile([P, SK], fp16, tag="esc", bufs=1)
            escs = sm.tile([P, SK], fp16, tag="escs", bufs=1)
            ssum2 = stats.tile([P, KC], f32, tag="ssum2")
            for kc in range(KC):
                nc.scalar.activation(esc[:, kc * 512:(kc + 1) * 512], lps[kc][:],
                                     AF.Exp, bias=negm[:, 0:1], scale=1.0)
                nc.vector.tensor_mul(escs[:, kc * 512:(kc + 1) * 512],
                                     esc[:, kc * 512:(kc + 1) * 512],
                                     E_sb[:, kc * 512:(kc + 1) * 512])
                nc.vector.reduce_sum(ssum2[:, kc:kc + 1],
                                     escs[:, kc * 512:(kc + 1) * 512], axis=AX)
            if qi + 1 < QI:
                if (qi + 1) % 4 == 0:
                    qxg = nxt_qxg
                cur = emit_logits(qxg, qi + 1)
            ssum = stats.tile([P, 1], f32, tag="ssum")
            nc.vector.reduce_sum(ssum[:], ssum2[:], axis=AX)
            msb = stats.tile([P, 2], f32, tag="msb")
            nc.vector.tensor_copy(msb[:, 0:1], negm[:])
            nc.vector.tensor_copy(msb[:, 1:2], ssum[:])
            nc.sync.dma_start(ms[qi * P:(qi + 1) * P, :], msb[:])

            escT = sm.tile([P, SK], fp16, tag="escT")
            for t4 in range(KC):
                tp = psT.tile([P, 512], fp16, tag="tpps")
                for j in range(4):
                    nc.tensor.matmul(tp[:, j * P:(j + 1) * P],
                                     escs[:, (t4 * 4 + j) * P:(t4 * 4 + j + 1) * P],
                                     ident[:], is_transpose=True,
                                     start=(j == 0), stop=(j == 3))
                nc.vector.tensor_copy(escT[:, t4 * 512:(t4 + 1) * 512], tp[:])

            for gc in range(2):
                pvps = psV.tile([P, 512], f32, tag="pvps")
                for kb in range(KB):
                    nc.tensor.matmul(pvps[:], escT[:, kb * P:(kb + 1) * P],
                                     vp[kb][:, gc * 512:(gc + 1) * 512],
                                     start=(kb == 0), stop=(kb == KB - 1))
                ob = outp.tile([P, 512], f32, tag="ob")
                nc.scalar.activation(ob[:], pvps[:], AF.Identity, scale=1.0)
                nc.sync.dma_start(
                    out[qi * P:(qi + 1) * P, gc * 512:(gc + 1) * 512], ob[:])

    nc.compile()
    return nc


def _round_f32r(x):
    xi = np.ascontiguousarray(x, dtype=np.float32).view(np.uint32)
    r = (xi + np.uint32(0x800)) & np.uint32(0xFFFFF000)
    return r.view(np.float32)


def _get_nc(repeat=1):
    key = f"nc{repeat}"
    if key not in _CACHE:
        _CACHE[key] = _build(repeat)
    return _CACHE[key]


def _make_in_maps(q, k, v, Wq, bq, Wk, bk, Wv, bv):
    q = np.ascontiguousarray(q, np.float32)
    k = np.ascontiguousarray(k, np.float32)
    v = np.ascontiguousarray(v, np.float32)
    Wq32 = np.ascontiguousarray(Wq, np.float32)
    Wk32 = np.ascontiguousarray(Wk, np.float32)
    bq32 = np.ascontiguousarray(bq, np.float32)
    bk32 = np.ascontiguousarray(bk, np.float32)
    # W* = Wq^T @ Wk ; device stationary layout needs W*^T = Wk^T @ Wq
    wsT = np.ascontiguousarray(Wk32.T @ Wq32).astype(np.float16)
    wvT = np.ascontiguousarray(np.float32(Wv).T).astype(np.float16)
    # per-key logit bias c[t] = bq . kp[t] = k[t] . (Wk^T bq) + bq.bk
    u = Wk32.T @ bq32
    beta = np.float32(bq32 @ bk32)
    qT = [np.ascontiguousarray(q[b].T).astype(np.float16) for b in range(B)]
    in_maps = []
    for c in range(N_CORES):
        b, h = divmod(c, 2)
        ksl = k[b, h * SK:(h + 1) * SK, :]
        kT_c = np.ascontiguousarray(ksl.T).astype(np.float16)
        vT_c = np.ascontiguousarray(v[b, h * SK:(h + 1) * SK, :].T).astype(np.float16)
        c_bias = (ksl @ u + beta).astype(np.float32)
        E_c = np.ascontiguousarray(
            np.broadcast_to(np.exp(c_bias)[None, :], (P, SK)), np.float32)
        in_maps.append({
            "qT": qT[b], "kT": kT_c, "vT": vT_c,
            "wsT": wsT, "wvT": wvT, "Eb": E_c,
        })
    return in_maps


def _execute(in_maps, trace=False):
    nc = _get_nc()
    return run_bass_kernel_spmd(nc, in_maps, list(range(N_CORES)), trace=trace)


def _merge(results, q, bv):
    """Flash-style merge of the two key-half partials per batch."""
    out = np.empty((B, S, F), np.float32)
    bv64 = np.asarray(bv, np.float64)
    for b in range(B):
        r0, r1 = results[2 * b], results[2 * b + 1]
        o0 = r0["out"].astype(np.float64)
        o1 = r1["out"].astype(np.float64)
        m0 = -r0["ms"][:, 0].astype(np.float64)
        m1 = -r1["ms"][:, 0].astype(np.float64)
        s0 = r0["ms"][:, 1].astype(np.float64)
        s1 = r1["ms"][:, 1].astype(np.float64)
        m = np.maximum(m0, m1)
        a0 = np.exp(m0 - m)
        a1 = np.exp(m1 - m)
        num = o0 * a0[:, None] + o1 * a1[:, None]
        den = s0 * a0 + s1 * a1
        out[b] = (num / den[:, None] + q[b].astype(np.float64) + bv64
                  ).astype(np.float32)
    return out


def kernel(q, k, v, Wq, bq, Wk, bk, Wv, bv):
    q = np.ascontiguousarray(q, np.float32)
    in_maps = _make_in_maps(q, k, v, Wq, bq, Wk, bk, Wv, bv)
    res = _execute(in_maps)
    return _merge(res.results, q, bv)



# revision 3
# speedup vs baseline: 1.5963x; 1.5963x over previous
"""Single-head attention (B=4, S=2048, F=1024) on 8 TRN2 NeuronCores.

All matmul operands are fp16 (e5m10: ~same 11-bit effective mantissa as
fp32r for the logits chain, far better than bf16 for the esc/vp side),
which halves every input DMA stream vs the f32r/bf16 mix while keeping
the same 1-col/cycle PE streaming rate. The Q projection is
algebraically fused into the key side:

  logits = (q Wq^T + bq)(k Wk^T + bk)^T
         = q (Wq^T Wk) k^T + (per-row const) + c[t],   c[t] = bq . kp[t]

The per-row term drops out of softmax, so with W* = Wq^T Wk and
ke = k @ W*^T the device computes logits0 = q . ke^T (raw q streamed
from DRAM, no Q projection matmuls at all) and applies the per-key bias
multiplicatively on the probabilities: esc' = exp(logits0 - m) * E,
E[t] = e^{c[t]} (host-precomputed, applied on the vector engine).

Key-split sharding: core c handles batch b=c//2 and KEY half h=c%2
(keys [h*1024, (h+1)*1024)), with ALL 2048 q rows. Each core emits an
UNNORMALIZED partial attention output plus per-row (max, sum) softmax
stats; the host merges the two halves flash-style:
  m = max(m0, m1); out = (o0*e^{m0-m} + o1*e^{m1-m}) /
                         (s0*e^{m0-m} + s1*e^{m1-m}) + q + bv
"""

import numpy as np
from contextlib import ExitStack

import concourse.bass as bass
import concourse.tile as tile
import concourse.mybir as mybir
from concourse import bacc
from concourse.bass_utils import run_bass_kernel_spmd
from concourse.masks import make_identity


B, S, F = 4, 2048, 1024
P = 128
SK = S // 2            # keys per core
FT = F // P            # 8 contraction tiles
GT = F // P            # 8 output-feature tiles
KC = SK // 512         # 2 key chunks of 512
QI = S // P            # 16 q-tiles per core
KB = SK // P           # 8 key blocks
N_CORES = 8

f32 = mybir.dt.float32
f32r = mybir.dt.float32r
bf16 = mybir.dt.bfloat16
fp16 = mybir.dt.float16
AX = mybir.AxisListType.X
AF = mybir.ActivationFunctionType
ALU = mybir.AluOpType

_CACHE = {}


def _build(repeat=1):
    nc = bacc.Bacc("TRN2", target_bir_lowering=False, debug=False,
                   num_devices=N_CORES)
    qT = nc.dram_tensor("qT", [F, S], fp16, kind="ExternalInput").ap()
    kT = nc.dram_tensor("kT", [F, SK], fp16, kind="ExternalInput").ap()
    vT = nc.dram_tensor("vT", [F, SK], fp16, kind="ExternalInput").ap()
    wsT = nc.dram_tensor("wsT", [F, F], fp16, kind="ExternalInput").ap()
    wvT = nc.dram_tensor("wvT", [F, F], fp16, kind="ExternalInput").ap()
    Eb = nc.dram_tensor("Eb", [P, SK], f32, kind="ExternalInput").ap()
    out = nc.dram_tensor("out", [S, F], f32, kind="ExternalOutput").ap()
    ms = nc.dram_tensor("ms", [S, 2], f32, kind="ExternalOutput").ap()

    with tile.TileContext(nc) as tc, ExitStack() as ctx:
      consts = ctx.enter_context(tc.tile_pool(name="consts", bufs=1))
      wpool = ctx.enter_context(tc.tile_pool(name="w", bufs=8))
      xin = ctx.enter_context(tc.tile_pool(name="xin", bufs=16))
      vxin = ctx.enter_context(tc.tile_pool(name="vxin", bufs=16))
      qx_pool = ctx.enter_context(tc.tile_pool(name="qx", bufs=2))
      proj = ctx.enter_context(tc.tile_pool(name="proj", bufs=1))
      sm = ctx.enter_context(tc.tile_pool(name="sm", bufs=2))
      stats = ctx.enter_context(tc.tile_pool(name="stats", bufs=2))
      outp = ctx.enter_context(tc.tile_pool(name="outp", bufs=2))
      psA = ctx.enter_context(tc.tile_pool(name="psA", bufs=4, space="PSUM"))
      psT = ctx.enter_context(tc.tile_pool(name="psT", bufs=2, space="PSUM"))
      psV = ctx.enter_context(tc.tile_pool(name="psV", bufs=2, space="PSUM"))
      for _rep in range(repeat):
        ident = consts.tile([P, P], fp16, tag="ident")
        make_identity(nc, ident)
        keT = [proj.tile([P, SK], fp16, tag=f"keT{g}", name=f"keT{g}")
               for g in range(GT)]
        vp = [proj.tile([P, F], fp16, tag=f"vp{i}", name=f"vp{i}")
              for i in range(KB)]

        # DMA issue order = need order: interleave ws and kx(sc=0) tiles so
        # the ft-outer first chunk can start after ws[0]+kx[0] land; wv/E/qx
        # follow, they are needed tens of us later.
        vx01 = [vxin.tile([P, 512], fp16, tag="vxin", name="vxin")
                for _ in range(2 * FT)]
        wv_sb = [wpool.tile([P, F], fp16, tag="wv", name="wv")
                 for _ in range(FT)]
        for ft in range(FT):
            nc.sync.dma_start(wv_sb[ft][:],
                              wvT[ft * P:(ft + 1) * P, :])
            nc.sync.dma_start(
                vx01[ft][:], vT[ft * P:(ft + 1) * P, 0:512])
        wsA = [wpool.tile([P, 512], fp16, tag="wsA", name="wsA")
               for _ in range(FT)]
        wsB = [wpool.tile([P, 512], fp16, tag="wsB", name="wsB")
               for _ in range(FT)]
        kx0 = [xin.tile([P, 512], fp16, tag="xin", name="xin")
               for _ in range(FT)]
        for ft in range(FT):
            nc.sync.dma_start(wsA[ft][:],
                              wsT[ft * P:(ft + 1) * P, 0:512])
            nc.sync.dma_start(
                kx0[ft][:],
                kT[ft * P:(ft + 1) * P, 0:512])
        for ft in range(FT):
            nc.sync.dma_start(wsB[ft][:],
                              wsT[ft * P:(ft + 1) * P, 512:1024])
        for ft in range(FT):
            nc.sync.dma_start(
                vx01[FT + ft][:], vT[ft * P:(ft + 1) * P, 512:1024])
        kx1 = [xin.tile([P, 512], fp16, tag="xin", name="xin")
               for _ in range(FT)]
        for ft in range(FT):
            nc.sync.dma_start(
                kx1[ft][:],
                kT[ft * P:(ft + 1) * P, 512:1024])
        E_sb = consts.tile([P, SK], f32, tag="Eb")
        nc.sync.dma_start(E_sb[:], Eb)

        # ---- projections, interleaved V/ke per key chunk so DMA
        # arrives ahead of need: V0 (3MB bf16), ke0 (ws+kx0 stream), V1, ke1.
        # All ft-outer in half-chunks of 4 psum tiles: first matmul needs only
        # the ft=0 tiles; the rest stream in behind compute, and the scalar
        # engine drains each half-chunk while the PE runs the next.
        def vproj_chunk(sc):
            vx = vx01[sc * FT:(sc + 1) * FT]
            for half in range(2):
                psh = [psA.tile([P, 512], f32, tag="mmps", name="psh")
                       for _ in range(4)]
                combos = [(half * 2 + b, gc) for b in range(2)
                          for gc in range(2)]
                for ft in range(FT):
                    for ci, (blk, gc) in enumerate(combos):
                        nc.tensor.matmul(
                            psh[ci][:], vx[ft][:, blk * P:(blk + 1) * P],
                            wv_sb[ft][:, gc * 512:(gc + 1) * 512],
                            start=(ft == 0), stop=(ft == FT - 1))
                for ci, (blk, gc) in enumerate(combos):
                    kb = sc * 4 + blk
                    nc.vector.tensor_copy(vp[kb][:, gc * 512:(gc + 1) * 512],
                                          psh[ci][:])

        def keproj_chunk(sc, kxc):
            for hc in range(2):
                wsh = wsA if hc == 0 else wsB
                psh = [psA.tile([P, 512], f32, tag="mmps", name="psh")
                       for _ in range(4)]
                for ft in range(FT):
                    for gi in range(4):
                        nc.tensor.matmul(psh[gi][:],
                                         wsh[ft][:, gi * P:(gi + 1) * P],
                                         kxc[ft][:], start=(ft == 0),
                                         stop=(ft == FT - 1))
                for gi in range(4):
                    gt = hc * 4 + gi
                    nc.scalar.activation(keT[gt][:, sc * 512:(sc + 1) * 512],
                                         psh[gi][:], AF.Identity, scale=1.0)

        vproj_chunk(0)
        keproj_chunk(0, kx0)
        vproj_chunk(1)
        keproj_chunk(1, kx1)

        # ---- attention over the local key half, pipelined over q-tiles ----
        # qx groups hold 4 q-tiles (2KB DMA lines); rotate via pool bufs=2.
        def load_qx(qg):
            qx = [qx_pool.tile([P, 512], fp16, tag=f"qx{ft}", name="qx")
                  for ft in range(FT)]
            for ft in range(FT):
                nc.sync.dma_start(
                    qx[ft][:],
                    qT[ft * P:(ft + 1) * P, qg * 512:(qg + 1) * 512])
            return qx

        def emit_logits(qx, qi):
            o = (qi % 4) * P
            lps = []
            m4 = stats.tile([P, KC], f32, tag="m4")
            for kc in range(KC):
                ps = psA.tile([P, 512], f32, tag="mmps")
                for gt in range(GT):
                    nc.tensor.matmul(ps[:], qx[gt][:, o:o + P],
                                     keT[gt][:, kc * 512:(kc + 1) * 512],
                                     start=(gt == 0), stop=(gt == GT - 1))
                nc.vector.reduce_max(m4[:, kc:kc + 1], ps[:], axis=AX)
                lps.append(ps)
            return lps, m4

        qxg = load_qx(0)
        nxt_qxg = None
        pend = None
        cur = emit_logits(qxg, 0)
        for qi in range(QI):
            if qi % 4 == 0 and qi // 4 + 1 < QI // 4:
                nxt_qxg = load_qx(qi // 4 + 1)
            lps, m4 = cur
            negm = stats.tile([P, 1], f32, tag="negm")
            nc.vector.reduce_max(negm[:], m4[:], axis=AX, negate=True)
            esc = sm.tile([P, SK], fp16, tag="esc", bufs=1)
            escs = sm.tile([P, SK], fp16, tag="escs", bufs=1)
            ssum2 = stats.tile([P, KC], f32, tag="ssum2")
            for kc in range(KC):
                nc.scalar.activation(esc[:, kc * 512:(kc + 1) * 512], lps[kc][:],
                                     AF.Exp, bias=negm[:, 0:1], scale=1.0)
                nc.vector.tensor_mul(escs[:, kc * 512:(kc + 1) * 512],
                                     esc[:, kc * 512:(kc + 1) * 512],
                                     E_sb[:, kc * 512:(kc + 1) * 512])
                nc.vector.reduce_sum(ssum2[:, kc:kc + 1],
                                     escs[:, kc * 512:(kc + 1) * 512], axis=AX)
            if qi + 1 < QI:
                if (qi + 1) % 4 == 0:
                    qxg = nxt_qxg
                cur = emit_logits(qxg, qi + 1)
            ssum = stats.tile([P, 1], f32, tag="ssum")
            nc.vector.reduce_sum(ssum[:], ssum2[:], axis=AX)
            msb = stats.tile([P, 2], f32, tag="msb")
            nc.vector.tensor_copy(msb[:, 0:1], negm[:])
            nc.vector.tensor_copy(msb[:, 1:2], ssum[:])
            nc.sync.dma_start(ms[qi * P:(qi + 1) * P, :], msb[:])

            escT = sm.tile([P, SK], fp16, tag="escT")
            for t4 in range(KC):
                tp = psT.tile([P, 512], fp16, tag="tpps")
                for j in range(4):
                    nc.tensor.matmul(tp[:, j * P:(j + 1) * P],
                                     escs[:, (t4 * 4 + j) * P:(t4 * 4 + j + 1) * P],
                                     ident[:], is_transpose=True,
                                     start=(j == 0), stop=(j == 3))
                nc.vector.tensor_copy(escT[:, t4 * 512:(t4 + 1) * 512], tp[:])

            for gc in range(2):
                pvps = psV.tile([P, 512], f32, tag="pvps")
                for kb in range(KB):
                    nc.tensor.matmul(pvps[:], escT[:, kb * P:(kb + 1) * P],
                                     vp[kb][:, gc * 512:(gc + 1) * 512],
                                     start=(kb == 0), stop=(kb == KB - 1))
                ob = outp.tile([P, 512], f32, tag="ob")
                nc.scalar.activation(ob[:], pvps[:], AF.Identity, scale=1.0)
                nc.sync.dma_start(
                    out[qi * P:(qi + 1) * P, gc * 512:(gc + 1) * 512], ob[:])

    nc.compile()
    return nc


def _round_f32r(x):
    xi = np.ascontiguousarray(x, dtype=np.float32).view(np.uint32)
    r = (xi + np.uint32(0x800)) & np.uint32(0xFFFFF000)
    return r.view(np.float32)


def _get_nc(repeat=1):
    key = f"nc{repeat}"
    if key not in _CACHE:
        _CACHE[key] = _build(repeat)
    return _CACHE[key]


def _make_in_maps(q, k, v, Wq, bq, Wk, bk, Wv, bv):
    q = np.ascontiguousarray(q, np.float32)
    k = np.ascontiguousarray(k, np.float32)
    v = np.ascontiguousarray(v, np.float32)
    Wq32 = np.ascontiguousarray(Wq, np.float32)
    Wk32 = np.ascontiguousarray(Wk, np.float32)
    bq32 = np.ascontiguousarray(bq, np.float32)
    bk32 = np.ascontiguousarray(bk, np.float32)
    # W* = Wq^T @ Wk ; device stationary layout needs W*^T = Wk^T @ Wq
    wsT = np.ascontiguousarray(Wk32.T @ Wq32).astype(np.float16)
    wvT = np.ascontiguousarray(np.float32(Wv).T).astype(np.float16)
    # per-key logit bias c[t] = bq . kp[t] = k[t] . (Wk^T bq) + bq.bk
    u = Wk32.T @ bq32
    beta = np.float32(bq32 @ bk32)
    qT = [np.ascontiguousarray(q[b].T).astype(np.float16) for b in range(B)]
    in_maps = []
    for c in range(N_CORES):
        b, h = divmod(c, 2)
        ksl = k[b, h * SK:(h + 1) * SK, :]
        kT_c = np.ascontiguousarray(ksl.T).astype(np.float16)
        vT_c = np.ascontiguousarray(v[b, h * SK:(h + 1) * SK, :].T).astype(np.float16)
        c_bias = (ksl @ u + beta).astype(np.float32)
        E_c = np.ascontiguousarray(
            np.broadcast_to(np.exp(c_bias)[None, :], (P, SK)), np.float32)
        in_maps.append({
            "qT": qT[b], "kT": kT_c, "vT": vT_c,
            "wsT": wsT, "wvT": wvT, "Eb": E_c,
        })
    return in_maps


def _execute(in_maps, trace=False):
    nc = _get_nc()
    return run_bass_kernel_spmd(nc, in_maps, list(range(N_CORES)), trace=trace)


def _merge(results, q, bv):
    """Flash-style merge of the two key-half partials per batch."""
    out = np.empty((B, S, F), np.float32)
    bv64 = np.asarray(bv, np.float64)
    for b in range(B):
        r0, r1 = results[2 * b], results[2 * b + 1]
        o0 = r0["out"].astype(np.float64)
        o1 = r1["out"].astype(np.float64)
        m0 = -r0["ms"][:, 0].astype(np.float64)
        m1 = -r1["ms"][:, 0].astype(np.float64)
        s0 = r0["ms"][:, 1].astype(np.float64)
        s1 = r1["ms"][:, 1].astype(np.float64)
        m = np.maximum(m0, m1)
        a0 = np.exp(m0 - m)
        a1 = np.exp(m1 - m)
        num = o0 * a0[:, None] + o1 * a1[:, None]
        den = s0 * a0 + s1 * a1
        out[b] = (num / den[:, None] + q[b].astype(np.float64) + bv64
                  ).astype(np.float32)
    return out


def kernel(q, k, v, Wq, bq, Wk, bk, Wv, bv):
    q = np.ascontiguousarray(q, np.float32)
    in_maps = _make_in_maps(q, k, v, Wq, bq, Wk, bk, Wv, bv)
    res = _execute(in_maps)
    return _merge(res.results, q, bv)

